# revision 1
# baseline (speedup 1.0000x reference)
"""Trainium2 Bass kernel for CustomStellarEncoder (GNN message passing).

8 NeuronCores, dst-sharded graph parallelism, v2 (batched-gather rewrite):
  - Stage 1a (per-core): own-shard s = x @ W1.T in [feat, node] layout from
    xps16; bn_stats on psum, raw s stashed fp16 into featT16. BN1 stats
    (sum/sumsq) AllReduced across cores; A1/B1 finalized; featT16 updated
    in place to relu(A1*s + B1); featd written fp16.
  - Stage 1b (replicated): full-node fp16 gather table [100352, 128] built
    NODE-major: per 128-node chunk, matmul(lhsT=x16 chunk with ones row,
    rhs=[A1-scaled W1T; B1 row]) -> psum [node, feat]; DVE relu -> fp16.
    Columns permuted per 1024-node group so each partition writes 2KB
    contiguous table bytes (8 consecutive rows).
  - Stage 2 (dst-sharded): edges sorted by dst, 128-dst blocks, chunk grid
    uniform across cores. Gathers BATCHED: one indirect DMA per block-group
    fetches [128, k, 128] fp16 rows (srcv offsets int32). Per 128-edge
    chunk: matmul(bps[feat,dst] += gt_chunk.T @ onehot(dst_local)), fp16.
    Finalize: rb = broadcast(recip degree) via K=1 matmul, aggT16 = bps*rb.
  - Stage 3: sageT = WlT.T @ aggT16 + WrT.T @ featT16 per 512 cols (fp16),
    BN2 stats with cross-core AllReduce, out = A2*sage + B2 fp16.

b1 and bl are dropped: both cancel exactly under the following BatchNorm.
Outputs are fp16 [feat, node] per shard; host casts/transposes/concats.
"""

from contextlib import ExitStack

import numpy as np

import concourse.bass as bass
import concourse.tile as tile
from concourse import bacc, mybir
from concourse.bass_utils import run_bass_kernel_spmd

N_NODES = 100000
N_EDGES = 1600000
IN_DIM = 48
HID = 128
BN_EPS = 1e-5
NCORES = 8
SHARD = N_NODES // NCORES          # 12500
P = 128
NBLK = (SHARD + P - 1) // P        # 98
NODE_PAD = NBLK * P                # 12544
NGRP = 98                          # 1024-node groups in padded table
NP2 = NGRP * 1024                  # 100352 padded table rows
HALF2 = NP2 // 2                   # 50176 (= 49 groups, group-aligned)
SHALF = SHARD // 2                 # 6250
BG = 2                             # dst blocks per batched gather
F32 = mybir.dt.float32
F16 = mybir.dt.float16
I32 = mybir.dt.int32
AX = mybir.AxisListType
ALU = mybir.AluOpType
ACTF = mybir.ActivationFunctionType


def _host_prep(x, edge_index, W1, Wl, Wr, g1, be1, g2, be2):
    # ---- x16: [128, HALF2] fp16, 2 bands of 48 + ones rows, columns
    # permuted so chunk i / partition p of group g is node 1024g + 8p + i.
    xpad = np.zeros((NP2, IN_DIM), np.float32)
    xpad[:N_NODES] = x
    j = np.arange(1024)
    nig = 8 * (j % P) + j // P                      # node-in-group per col
    perm = (np.arange(0, NP2, 1024)[:, None] + nig[None, :]).ravel()
    xperm = xpad[perm]                              # [NP2, 48] col-ordered
    x16 = np.zeros((P, HALF2), np.float16)
    x16[0:IN_DIM] = xperm[:HALF2].T
    x16[IN_DIM] = 1.0
    x16[64:64 + IN_DIM] = xperm[HALF2:].T
    x16[64 + IN_DIM] = 1.0

    w1tp = np.zeros((P, P), np.float16)
    w1tp[0:IN_DIM] = W1.T
    w1tp[64:64 + IN_DIM] = W1.T

    msk = np.zeros((P, P), np.float16)
    msk[IN_DIM] = 1.0
    msk[64 + IN_DIM] = 1.0

    wlwr = np.zeros((P, 2 * P), np.float16)
    wlwr[:, 0:P] = Wl.T
    wlwr[:, P:2 * P] = Wr.T

    cvec = np.zeros((P, 4), np.float32)
    cvec[:, 0] = g1
    cvec[:, 1] = be1
    cvec[:, 2] = g2
    cvec[:, 3] = be2

    # ---- edges sorted by (core, dst) ----
    src = np.asarray(edge_index[0], np.int64)
    dst = np.asarray(edge_index[1], np.int64)
    core_of = dst // SHARD
    order = np.argsort(core_of * N_NODES + dst, kind="stable")
    src_s, dst_s = src[order], dst[order]
    core_starts = np.searchsorted(core_of[order], np.arange(NCORES + 1))

    nbc = np.zeros((NCORES, NBLK), np.int64)
    per_core = []
    for c in range(NCORES):
        s, e = int(core_starts[c]), int(core_starts[c + 1])
        dl = dst_s[s:e] - c * SHARD          # local dst, sorted ascending
        nbc[c] = np.bincount(dl // P, minlength=NBLK)
        per_core.append((src_s[s:e], dl))
    cpb = np.maximum(1, (nbc.max(axis=0) + P - 1) // P).astype(np.int64)
    offs = np.concatenate([[0], np.cumsum(cpb)]).astype(np.int64)
    C = int(offs[-1])

    srcv = np.zeros((NCORES, P, C), np.int32)
    dstv = np.full((NCORES, P, C), -1.0, np.float16)
    recn = np.zeros((NCORES, 1, NODE_PAD), np.float16)
    xps = np.zeros((NCORES, P, SHALF), np.float16)
    for c in range(NCORES):
        s_arr, dl = per_core[c]
        bstart = np.concatenate([[0], np.cumsum(nbc[c])])
        for b in range(NBLK):
            n = int(nbc[c][b])
            if n == 0:
                continue
            e0 = int(bstart[b])
            idx = np.arange(n)
            srcv[c, idx % P, offs[b] + idx // P] = s_arr[e0:e0 + n]
            dstv[c, idx % P, offs[b] + idx // P] = \
                (dl[e0:e0 + n] - b * P).astype(np.float16)
        cnt = np.bincount(dl, minlength=NODE_PAD).astype(np.float32)
        recn[c, 0] = (1.0 / np.maximum(cnt, 1.0)).astype(np.float16)
        base = c * SHARD
        xps[c, 0:IN_DIM] = x[base:base + SHALF].T
        xps[c, 64:64 + IN_DIM] = x[base + SHALF:base + SHARD].T
    return x16, w1tp, msk, wlwr, cvec, srcv, dstv, recn, xps, cpb, offs, C


def _build(nc, cpb, offs, C):
    cpbmax = int(cpb.max())
    x16d = nc.dram_tensor("x16", [P, HALF2], F16, kind="ExternalInput")
    xpsd = nc.dram_tensor("xps", [P, SHALF], F16, kind="ExternalInput")
    w1d = nc.dram_tensor("w1tp", [P, P], F16, kind="ExternalInput")
    wld = nc.dram_tensor("wlwr", [P, 2 * P], F16, kind="ExternalInput")
    cvd = nc.dram_tensor("cvec", [P, 4], F32, kind="ExternalInput")
    srd = nc.dram_tensor("srcv", [P, C], I32, kind="ExternalInput")
    dsd = nc.dram_tensor("dstv", [P, C], F16, kind="ExternalInput")
    rcd = nc.dram_tensor("recn", [1, NODE_PAD], F16, kind="ExternalInput")
    mkd = nc.dram_tensor("msk", [P, P], F16, kind="ExternalInput")
    featd = nc.dram_tensor("featd", [P, SHARD], F16, kind="ExternalOutput")
    outfd = nc.dram_tensor("outfd", [P, SHARD], F16, kind="ExternalOutput")

    with tile.TileContext(nc) as tc, ExitStack() as ctx:
        persist = ctx.enter_context(tc.tile_pool(name="persist", bufs=1))
        dram = ctx.enter_context(tc.tile_pool(name="dram", bufs=1, space="DRAM"))
        xpool = ctx.enter_context(tc.tile_pool(name="xpool", bufs=3))
        twpool = ctx.enter_context(tc.tile_pool(name="twpool", bufs=3))
        gpool = ctx.enter_context(tc.tile_pool(name="gpool", bufs=40))
        ohpool = ctx.enter_context(tc.tile_pool(name="ohpool", bufs=3))
        opool = ctx.enter_context(tc.tile_pool(name="opool", bufs=3))
        psum = ctx.enter_context(tc.tile_pool(name="psum", bufs=1, space="PSUM"))

        table = dram.tile([NP2, P], F16)
        bn1_in = dram.tile([P, 2], F32)
        bn1_out = dram.tile([P, 2], F32, addr_space="Shared")
        bn2_in = dram.tile([P, 2], F32)
        bn2_out = dram.tile([P, 2], F32, addr_space="Shared")

        # ---- constants ----
        w1t16 = persist.tile([P, P], F16)
        nc.sync.dma_start(w1t16[:], w1d[:])
        wlwr16 = persist.tile([P, 2 * P], F16)
        nc.sync.dma_start(wlwr16[:], wld[:])
        cv = persist.tile([P, 4], F32)
        nc.sync.dma_start(cv[:], cvd[:])
        srct = persist.tile([P, C], I32)
        nc.sync.dma_start(srct[:], srd[:])
        dstt = persist.tile([P, C], F16)
        nc.sync.dma_start(dstt[:], dsd[:])
        recn16 = persist.tile([1, NODE_PAD], F16)
        nc.sync.dma_start(recn16[:], rcd[:])
        msk16 = persist.tile([P, P], F16)
        nc.sync.dma_start(msk16[:], mkd[:])

        id32 = persist.tile([P, P], F32)
        from concourse.masks import make_identity
        make_identity(nc, id32[:])
        ones1 = persist.tile([1, P], F16)
        nc.vector.memset(ones1[:], 1.0)
        iota2 = persist.tile([P, P], F32)
        nc.gpsimd.iota(iota2[:], pattern=[[1, P]], base=0, channel_multiplier=0,
                       allow_small_or_imprecise_dtypes=True)
        iotab = persist.tile([P, cpbmax, P], F16)
        for s in range(cpbmax):
            nc.vector.tensor_copy(iotab[:, s:s + 1, :], iota2[:])

        featT16 = persist.tile([P, NODE_PAD], F16)
        nc.vector.memset(featT16[:, SHARD:NODE_PAD], 0.0)
        aggT16 = persist.tile([P, NODE_PAD], F16)
        sageT16 = persist.tile([P, NODE_PAD], F16)

        # ============ Stage 1a: own-shard raw s + BN1 partial stats ======
        SCH = (SHALF + 511) // 512  # 13
        st1 = persist.tile([P, 2 * SCH, 6], F32)
        for k in range(SCH):
            w = min(512, SHALF - 512 * k)
            xt = xpool.tile([P, 512], F16, tag="xt")
            nc.sync.dma_start(xt[:, :w], xpsd[:, 512 * k:512 * k + w])
            for h, (p0, slot) in enumerate(((0, 2 * k), (64, 2 * k + 1))):
                col0 = 512 * k + (0 if h == 0 else SHALF)
                ps = psum.tile([P, 512], F32, space="PSUM", tag="ps1", bufs=2)
                nc.tensor.matmul(ps[:, :w], lhsT=w1t16[p0:p0 + IN_DIM, :],
                                 rhs=xt[p0:p0 + IN_DIM, :w],
                                 start=True, stop=True)
                nc.vector.bn_stats(st1[:, slot:slot + 1, :], ps[:, :w])
                nc.scalar.copy(featT16[:, col0:col0 + w], ps[:, :w])

        # ---- BN1: local stats -> sum/sumsq -> AllReduce -> A1, B1 ----
        mv1 = persist.tile([P, 2], F32)
        nc.vector.bn_aggr(mv1[:], st1[:])
        ss1 = persist.tile([P, 2], F32)
        nc.scalar.mul(ss1[:, 0:1], mv1[:, 0:1], float(SHARD))
        nc.vector.tensor_tensor(ss1[:, 1:2], mv1[:, 0:1], mv1[:, 0:1],
                                op=ALU.mult)
        nc.vector.tensor_tensor(ss1[:, 1:2], mv1[:, 1:2], ss1[:, 1:2],
                                op=ALU.add)
        nc.vector.tensor_scalar_mul(ss1[:, 1:2], ss1[:, 1:2], float(SHARD))
        nc.sync.dma_start(bn1_in[:], ss1[:])
        nc.gpsimd.collective_compute(
            "AllReduce", ALU.add, replica_groups=[list(range(NCORES))],
            ins=[bn1_in[:]], outs=[bn1_out[:]])
        gst1 = persist.tile([P, 2], F32)
        nc.sync.dma_start(gst1[:], bn1_out[:])

        stat1 = persist.tile([P, 8], F32)
        nc.scalar.mul(stat1[:, 2:3], gst1[:, 0:1], 1.0 / N_NODES)   # mu1
        nc.scalar.mul(stat1[:, 3:4], gst1[:, 1:2], 1.0 / N_NODES)   # E[s^2]
        nc.vector.tensor_tensor(stat1[:, 4:5], stat1[:, 2:3], stat1[:, 2:3],
                                op=ALU.mult)
        nc.vector.tensor_tensor(stat1[:, 4:5], stat1[:, 3:4], stat1[:, 4:5],
                                op=ALU.subtract)
        nc.vector.tensor_scalar_add(stat1[:, 4:5], stat1[:, 4:5], BN_EPS)
        nc.vector.reciprocal(stat1[:, 5:6], stat1[:, 4:5])
        nc.scalar.sqrt(stat1[:, 5:6], stat1[:, 5:6])                # rsqrt
        A1 = persist.tile([P, 2], F32)   # col0 = A1, col1 = B1
        nc.vector.tensor_tensor(A1[:, 0:1], stat1[:, 5:6], cv[:, 0:1],
                                op=ALU.mult)
        nc.vector.tensor_tensor(A1[:, 1:2], stat1[:, 2:3], A1[:, 0:1],
                                op=ALU.mult)
        nc.vector.tensor_tensor(A1[:, 1:2], cv[:, 1:2], A1[:, 1:2],
                                op=ALU.subtract)

        # featT16 <- relu(A1 * s + B1) in place; write featd
        for k in range((SHARD + 511) // 512):
            w = min(512, SHARD - 512 * k)
            sl = slice(512 * k, 512 * k + w)
            nc.scalar.activation(featT16[:, sl], featT16[:, sl], ACTF.Relu,
                                 bias=A1[:, 1:2], scale=A1[:, 0:1])
        nc.sync.dma_start(featd[:], featT16[:, 0:SHARD])

        # ---- transpose [A1|B1] -> rows, broadcast, fold into weights ----
        arow_ps = psum.tile([1, P], F32, space="PSUM", tag="pc", bufs=2)
        nc.tensor.matmul(arow_ps[:], lhsT=A1[:, 0:1], rhs=id32[:],
                         is_transpose=True, start=True, stop=True)
        brow_ps = psum.tile([1, P], F32, space="PSUM", tag="pc", bufs=2)
        nc.tensor.matmul(brow_ps[:], lhsT=A1[:, 1:2], rhs=id32[:],
                         is_transpose=True, start=True, stop=True)
        arow16 = persist.tile([1, P], F16)
        brow16 = persist.tile([1, P], F16)
        nc.vector.tensor_copy(arow16[:], arow_ps[:])
        nc.vector.tensor_copy(brow16[:], brow_ps[:])
        a1b_ps = psum.tile([P, P], F32, space="PSUM", tag="pc", bufs=2)
        nc.tensor.matmul(a1b_ps[:], lhsT=ones1[0:1, :], rhs=arow16[:],
                         start=True, stop=True)
        b1b_ps = psum.tile([P, P], F32, space="PSUM", tag="pc", bufs=2)
        nc.tensor.matmul(b1b_ps[:], lhsT=ones1[0:1, :], rhs=brow16[:],
                         start=True, stop=True)
        # w1ts = W1T*A1 on data rows; B1 on the two ones-rows (msk selects).
        w1ts = persist.tile([P, P], F16)
        nc.vector.tensor_tensor(w1ts[:], w1t16[:], a1b_ps[:], op=ALU.mult)
        b1m = persist.tile([P, P], F16)
        nc.vector.tensor_tensor(b1m[:], msk16[:], b1b_ps[:], op=ALU.mult)
        nc.vector.tensor_tensor(w1ts[:], w1ts[:], b1m[:], op=ALU.add)

        # ============ Stage 1b: fp16 table, node-major ============
        for g in range(NGRP):
            h = 0 if g < NGRP // 2 else 64
            c0 = 1024 * (g if h == 0 else g - NGRP // 2)
            xg = xpool.tile([P, 1024], F16, tag="xg")
            nc.sync.dma_start(xg[:], x16d[:, c0:c0 + 1024])
            tw = twpool.tile([P, 8, P], F16, tag="tw")
            for i in range(8):
                pc = psum.tile([P, P], F32, space="PSUM", tag="pc", bufs=2)
                nc.tensor.matmul(pc[:], lhsT=xg[h:h + IN_DIM + 1,
                                                128 * i:128 * (i + 1)],
                                 rhs=w1ts[h:h + IN_DIM + 1, :],
                                 start=True, stop=True)
                nc.vector.tensor_scalar_max(tw[:, i, :], pc[:], 0.0)
            eng = nc.scalar if g % 2 else nc.sync
            eng.dma_start(
                table[1024 * g:1024 * (g + 1), :].rearrange(
                    "(p i) f -> p i f", p=P, i=8),
                tw[:])

        # ============ Stage 2: batched gather + one-hot aggregation ======
        # recb16[p, n] = 1/deg(n) broadcast across partitions (SBUF so the
        # finalize tensor_tensor has a single PSUM operand).
        recb16 = persist.tile([P, NODE_PAD], F16)
        for k in range(NSCH0 := (NODE_PAD + 511) // 512):
            w = min(512, NODE_PAD - 512 * k)
            rp = psum.tile([P, 512], F32, space="PSUM", tag="ps1", bufs=2)
            nc.tensor.matmul(rp[:, :w], lhsT=ones1[0:1, :],
                             rhs=recn16[0:1, 512 * k:512 * k + w],
                             start=True, stop=True)
            nc.scalar.copy(recb16[:, 512 * k:512 * k + w], rp[:, :w])

        table_ap = table[:]
        # Per-chunk [P,1] indirect gathers (HW-proven construct): one gather
        # per 128-edge chunk into a fresh [P,P] fp16 tile, consumed as the
        # matmul's stationary operand. gpool depth keeps the GpSimd queue fed.
        ohbs = {}

        def build_ohb(b):
            nch = int(cpb[b])
            off = int(offs[b])
            ohb = ohpool.tile([P, cpbmax, P], F16, tag="ohb", name=f"ohb{b}")
            nc.vector.tensor_tensor(
                ohb[:, :nch, :],
                dstt[:, off:off + nch].to_broadcast([P, nch, P]),
                iotab[:, :nch, :], op=ALU.is_equal)
            ohbs[b] = ohb

        bpss = {}

        def finalize(b):
            bps = bpss.pop(b)
            nc.vector.tensor_tensor(aggT16[:, P * b:P * (b + 1)],
                                    bps[:], recb16[:, P * b:P * (b + 1)],
                                    op=ALU.mult)

        build_ohb(0)
        if NBLK > 1:
            build_ohb(1)
        for b in range(NBLK):
            nch = int(cpb[b])
            off = int(offs[b])
            if b + 2 < NBLK:
                build_ohb(b + 2)
            ohb = ohbs.pop(b)
            bps = psum.tile([P, P], F32, space="PSUM", tag="bps", bufs=3,
                            name=f"bps{b}")
            bpss[b] = bps
            for j in range(nch):
                gt = gpool.tile([P, P], F16, tag="gt")
                nc.gpsimd.indirect_dma_start(
                    out=gt[:], out_offset=None, in_=table_ap,
                    in_offset=bass.IndirectOffsetOnAxis(
                        ap=srct[:, off + j:off + j + 1], axis=0))
                nc.tensor.matmul(bps[:], lhsT=gt[:], rhs=ohb[:, j, :],
                                 start=(j == 0), stop=(j == nch - 1))
            if b >= 1:
                finalize(b - 1)
        finalize(NBLK - 1)

        # ================= Stage 3: sage + BN2 =================
        NSCH = (NODE_PAD + 511) // 512  # 25 chunks (last = 256)
        st2 = persist.tile([P, NSCH, 6], F32)
        for k in range(NSCH):
            w = min(512, NODE_PAD - 512 * k)
            ws = min(512, max(0, SHARD - 512 * k))   # stats over 12500 only
            ps = psum.tile([P, 512], F32, space="PSUM", tag="ps1", bufs=2)
            nc.tensor.matmul(ps[:, :w], lhsT=wlwr16[:, 0:P],
                             rhs=aggT16[:, 512 * k:512 * k + w],
                             start=True, stop=False)
            nc.tensor.matmul(ps[:, :w], lhsT=wlwr16[:, P:2 * P],
                             rhs=featT16[:, 512 * k:512 * k + w],
                             start=False, stop=True)
            if ws > 0:
                nc.vector.bn_stats(st2[:, k:k + 1, :], ps[:, :ws])
            nc.scalar.copy(sageT16[:, 512 * k:512 * k + w], ps[:, :w])

        mv2 = persist.tile([P, 2], F32)
        nc.vector.bn_aggr(mv2[:], st2[:])
        ss2 = persist.tile([P, 2], F32)
        nc.scalar.mul(ss2[:, 0:1], mv2[:, 0:1], float(SHARD))
        nc.vector.tensor_tensor(ss2[:, 1:2], mv2[:, 0:1], mv2[:, 0:1],
                                op=ALU.mult)
        nc.vector.tensor_tensor(ss2[:, 1:2], mv2[:, 1:2], ss2[:, 1:2],
                                op=ALU.add)
        nc.vector.tensor_scalar_mul(ss2[:, 1:2], ss2[:, 1:2], float(SHARD))
        nc.sync.dma_start(bn2_in[:], ss2[:])
        nc.gpsimd.collective_compute(
            "AllReduce", ALU.add, replica_groups=[list(range(NCORES))],
            ins=[bn2_in[:]], outs=[bn2_out[:]])
        gst2 = persist.tile([P, 2], F32)
        nc.sync.dma_start(gst2[:], bn2_out[:])

        stat2 = persist.tile([P, 8], F32)
        nc.scalar.mul(stat2[:, 2:3], gst2[:, 0:1], 1.0 / N_NODES)   # mu2
        nc.scalar.mul(stat2[:, 3:4], gst2[:, 1:2], 1.0 / N_NODES)   # E[s^2]
        nc.vector.tensor_tensor(stat2[:, 4:5], stat2[:, 2:3], stat2[:, 2:3],
                                op=ALU.mult)
        nc.vector.tensor_tensor(stat2[:, 4:5], stat2[:, 3:4], stat2[:, 4:5],
                                op=ALU.subtract)
        nc.vector.tensor_scalar_add(stat2[:, 4:5], stat2[:, 4:5], BN_EPS)
        nc.vector.reciprocal(stat2[:, 5:6], stat2[:, 4:5])
        nc.scalar.sqrt(stat2[:, 5:6], stat2[:, 5:6])
        A2 = persist.tile([P, 2], F32)
        nc.vector.tensor_tensor(A2[:, 0:1], stat2[:, 5:6], cv[:, 2:3],
                                op=ALU.mult)
        nc.vector.tensor_tensor(A2[:, 1:2], stat2[:, 2:3], A2[:, 0:1],
                                op=ALU.mult)
        nc.vector.tensor_tensor(A2[:, 1:2], cv[:, 3:4], A2[:, 1:2],
                                op=ALU.subtract)

        for k in range((SHARD + 511) // 512):
            ws = min(512, SHARD - 512 * k)
            ot = opool.tile([P, 512], F16, tag="ot")
            nc.scalar.activation(ot[:, :ws], sageT16[:, 512 * k:512 * k + ws],
                                 ACTF.Identity, bias=A2[:, 1:2],
                                 scale=A2[:, 0:1])
            nc.sync.dma_start(outfd[:, 512 * k:512 * k + ws], ot[:, :ws])


def kernel(**inputs):
    x = np.asarray(inputs["x"], np.float32)
    edge_index = np.asarray(inputs["edge_index"])
    args = [x, edge_index,
            np.asarray(inputs["W1"], np.float32),
            np.asarray(inputs["Wl"], np.float32),
            np.asarray(inputs["Wr"], np.float32),
            np.asarray(inputs["g1"], np.float32),
            np.asarray(inputs["be1"], np.float32),
            np.asarray(inputs["g2"], np.float32),
            np.asarray(inputs["be2"], np.float32)]
    (x16, w1tp, msk, wlwr, cvec, srcv, dstv, recn, xps, cpb, offs, C) = \
        _host_prep(*args)

    nc = bacc.Bacc("TRN2", target_bir_lowering=False, debug=False,
                   num_devices=NCORES)
    _build(nc, cpb, offs, C)
    nc.compile()

    in_maps = []
    for c in range(NCORES):
        in_maps.append({
            "x16": x16, "xps": np.ascontiguousarray(xps[c]),
            "w1tp": w1tp, "msk": msk, "wlwr": wlwr, "cvec": cvec,
            "srcv": np.ascontiguousarray(srcv[c]),
            "dstv": np.ascontiguousarray(dstv[c]),
            "recn": np.ascontiguousarray(recn[c]),
        })
    res = run_bass_kernel_spmd(nc, in_maps, core_ids=list(range(NCORES)))
    feat = np.concatenate(
        [res.results[c]["featd"].astype(np.float32).T for c in range(NCORES)],
        axis=0)
    out_feat = np.concatenate(
        [res.results[c]["outfd"].astype(np.float32).T for c in range(NCORES)],
        axis=0)
    return (np.ascontiguousarray(feat), np.ascontiguousarray(out_feat))



# revision 16
# speedup vs baseline: 2.4082x; 2.4082x over previous
"""Trainium2 Bass kernel for CustomStellarEncoder (GNN message passing).

8 NeuronCores, dst-sharded graph parallelism, v3 (dma_gather rewrite):
  - Stage 1a (per-core): own-shard s = x @ W1.T in [feat, node] layout from
    xps16; bn_stats on psum, raw s stashed fp16 into featT16. BN1 stats
    (sum/sumsq) AllReduced across cores; A1/B1 finalized; featT16 updated
    in place to relu(A1*s + B1); featd written fp16.
  - Stage 1b (replicated): full-node fp16 gather table [100352, 128] built
    NODE-major (table row == node id); BN1 folded into weights, relu on DVE.
  - Stage 2 (dst-sharded): edges sorted by (super=dst//1024, range=src>>15,
    dst). Gathers via InstDMAGatherAnt (mlp ucode): 1024-idx int16 gathers
    round-robined over 4 SWDGE queues (per-queue descriptor ring budget
    ~64K; no mid-kernel reclaim on HW). Per 128-edge chunk: one-hot(dst)
    built by DVE is_equal against iota, matmul accumulates psum[feat, dst]
    per 128-dst block; supers of 8 blocks keep 8 psums live across the 4
    range-subruns. Finalize: aggT16 = psum * recip-degree broadcast.
  - Stage 3: sageT = WlT.T @ aggT16 + WrT.T @ featT16 per 512 cols (fp16),
    BN2 stats with cross-core AllReduce, out = A2*sage + B2 fp16 (sage
    written back into aggT16 columns to save SBUF).

b1 and bl are dropped: both cancel exactly under the following BatchNorm.
Outputs are fp16 [feat, node] per shard; host casts/transposes/concats.
"""

from contextlib import ExitStack

import numpy as np

import concourse.bass as bass
import concourse.tile as tile
from concourse import bacc, mybir
from concourse.bass_utils import run_bass_kernel_spmd
from concourse.library_config import mlp

N_NODES = 100000
N_EDGES = 1600000
IN_DIM = 48
HID = 128
BN_EPS = 1e-5
NCORES = 8
SHARD = N_NODES // NCORES          # 12500
P = 128
NBLK = (SHARD + P - 1) // P        # 98
NODE_PAD = NBLK * P                # 12544
NGRP = 98                          # 1024-node groups in padded table
NP2 = NGRP * 1024                  # 100352 padded table rows
HALF2 = NP2 // 2                   # 50176 (= 49 groups, group-aligned)
SHALF = SHARD // 2                 # 6250
F32 = mybir.dt.float32
F16 = mybir.dt.float16
I16 = mybir.dt.int16
AX = mybir.AxisListType
ALU = mybir.AluOpType
ACTF = mybir.ActivationFunctionType

SUPB = 8                            # dst blocks per super
NSUP = (NBLK + SUPB - 1) // SUPB    # 13 (last super has 2 blocks)
NRNG = 4
RBASE = [0, 32768, 65536, 98304]
RSIZE = [32768, 32768, 32768, NP2 - 98304]
GIDX = 1024                         # idxs per full dma_gather (8 chunks)
NQ = 4                              # SWDGE queues


def _host_prep(x, edge_index, W1, Wl, Wr, g1, be1, g2, be2):
    # ---- x16: [128, HALF2] fp16, 2 bands of 48 + ones rows, columns
    # permuted so chunk i / partition p of group g is node 1024g + 8p + i.
    xpad = np.zeros((NP2, IN_DIM), np.float32)
    xpad[:N_NODES] = x
    j = np.arange(1024)
    nig = 8 * (j % P) + j // P                      # node-in-group per col
    perm = (np.arange(0, NP2, 1024)[:, None] + nig[None, :]).ravel()
    xperm = xpad[perm]                              # [NP2, 48] col-ordered
    x16 = np.zeros((P, HALF2), np.float16)
    x16[0:IN_DIM] = xperm[:HALF2].T
    x16[IN_DIM] = 1.0
    x16[64:64 + IN_DIM] = xperm[HALF2:].T
    x16[64 + IN_DIM] = 1.0

    w1tp = np.zeros((P, P), np.float16)
    w1tp[0:IN_DIM] = W1.T
    w1tp[64:64 + IN_DIM] = W1.T

    msk = np.zeros((P, P), np.float16)
    msk[IN_DIM] = 1.0
    msk[64 + IN_DIM] = 1.0

    wlwr = np.zeros((P, 2 * P), np.float16)
    wlwr[:, 0:P] = Wl.T
    wlwr[:, P:2 * P] = Wr.T

    cvec = np.zeros((P, 4), np.float32)
    cvec[:, 0] = g1
    cvec[:, 1] = be1
    cvec[:, 2] = g2
    cvec[:, 3] = be2

    idm = np.eye(P, dtype=np.float32)
    iotas = np.zeros((P, SUPB, P), np.float16)
    iotas[:] = (128 * np.arange(SUPB)[:, None]
                + np.arange(P)[None, :]).astype(np.float16)[None, :, :]

    # ---- edges sorted by (core, super, range, dst) ----
    src = np.asarray(edge_index[0], np.int64)
    dst = np.asarray(edge_index[1], np.int64)
    core_of = dst // SHARD
    rng_of = src >> 15                    # 0..3 (98304.. -> 3)
    dl_all = dst - core_of * SHARD
    sup_of = dl_all // 1024
    key = ((core_of * NSUP + sup_of) * NRNG + rng_of) * (1 << 17) + dl_all
    order = np.argsort(key, kind="stable")
    src_s, dst_s, core_s = src[order], dst[order], core_of[order]
    rng_s, sup_s = rng_of[order], sup_of[order]
    dl_s = dl_all[order]

    core_starts = np.searchsorted(core_s, np.arange(NCORES + 1))

    # per (core, super, range) edge counts
    ncsr = np.zeros((NCORES, NSUP, NRNG), np.int64)
    flat = (core_s * NSUP + sup_s) * NRNG + rng_s
    bc = np.bincount(flat, minlength=NCORES * NSUP * NRNG)
    ncsr = bc.reshape(NCORES, NSUP, NRNG)
    cpsr = np.maximum(0, (ncsr.max(axis=0) + P - 1) // P)   # [NSUP, NRNG]
    CH = int(cpsr.sum())                                     # total chunks
    chof = np.zeros((NSUP, NRNG), np.int64)
    acc = 0
    for s in range(NSUP):
        for r in range(NRNG):
            chof[s, r] = acc
            acc += int(cpsr[s, r])

    # per-core wrapped int16 idxs + fp16 dst values (pad: idx 0, dst -1)
    srcw = np.zeros((NCORES, P, CH * 8), np.int16)
    dstw = np.full((NCORES, P, CH), -1.0, np.float16)
    # per-core per (s,r,chunk) block windows for union schedule
    blo = np.full((NSUP, NRNG, int(cpsr.max()) if CH else 1, ), 99, np.int64)
    bhi = np.full_like(blo, -1)
    recn = np.zeros((NCORES, 1, NODE_PAD), np.float16)
    xps = np.zeros((NCORES, P, SHALF), np.float16)
    for c in range(NCORES):
        s0, e0 = int(core_starts[c]), int(core_starts[c + 1])
        dl_c = dl_s[s0:e0]
        cnt = np.bincount(dl_c, minlength=NODE_PAD).astype(np.float32)
        recn[c, 0] = (1.0 / np.maximum(cnt, 1.0)).astype(np.float16)
        base = c * SHARD
        xps[c, 0:IN_DIM] = x[base:base + SHALF].T
        xps[c, 64:64 + IN_DIM] = x[base + SHALF:base + SHARD].T
        # run boundaries within this core
        sub = (sup_s[s0:e0] * NRNG + rng_s[s0:e0])
        starts = np.searchsorted(sub, np.arange(NSUP * NRNG + 1))
        for s in range(NSUP):
            for r in range(NRNG):
                a = int(starts[s * NRNG + r])
                b = int(starts[s * NRNG + r + 1])
                n = b - a
                if n == 0:
                    continue
                co = int(chof[s, r])
                i = np.arange(n)
                sv = (src_s[s0 + a:s0 + b] - RBASE[r]).astype(np.int16)
                dv = (dl_s[s0 + a:s0 + b] - 1024 * s).astype(np.float16)
                # idx wrap: idx position i -> [i % 16 (+16g), 8*co + i//16]
                colw = 8 * co + i // 16
                srcw[c, i % 16, colw] = sv
                dstw[c, i % P, co + i // P] = dv
                # block windows per chunk
                bb = (dl_s[s0 + a:s0 + b] - 1024 * s) // P
                for jj in range(int((n + P - 1) // P)):
                    seg = bb[jj * P:(jj + 1) * P]
                    blo[s, r, jj] = min(blo[s, r, jj], int(seg.min()))
                    bhi[s, r, jj] = max(bhi[s, r, jj], int(seg.max()))
    # replicate idx wrap across the 8 gpsimd core groups
    for g in range(1, 8):
        srcw[:, 16 * g:16 * g + 16, :] = srcw[:, 0:16, :]

    # ---- static schedule ----
    # per super: ordered list over (r, j) of (chunk_col, r, window lo, hi)
    # plus start/stop bookkeeping per block.
    sched = []          # [NSUP] -> list of (col, r, lo, hi)
    for s in range(NSUP):
        items = []
        for r in range(NRNG):
            for jj in range(int(cpsr[s, r])):
                lo, hi = int(blo[s, r, jj]), int(bhi[s, r, jj])
                if hi < 0:      # no core has edges in this chunk (all pad)
                    lo = hi = 0  # harmless zero matmul into block 0
                items.append((int(chof[s, r]) + jj, r, lo, hi))
        sched.append(items)

    # gather split per (s, r): list of (idx_col0, nchunks, first_chunk_col)
    gathers = []
    for s in range(NSUP):
        for r in range(NRNG):
            nch = int(cpsr[s, r])
            co = int(chof[s, r])
            jj = 0
            while jj < nch:
                k = min(8, nch - jj)
                gathers.append((s, r, 8 * (co + jj), k, co + jj))
                jj += k
    # ring budget check: pow2ceil(k) slot pages per gather, <=512/queue
    def p2(k):
        v = 1
        while v < k:
            v *= 2
        return v
    pages = [0] * NQ
    for i, (s, r, c0, k, cc) in enumerate(gathers):
        pages[i % NQ] += p2(k)
    assert max(pages) <= 448, f"SWDGE ring budget exceeded: {pages}"

    return (x16, w1tp, msk, wlwr, cvec, idm, iotas, srcw, dstw, recn, xps,
            cpsr, chof, CH, sched, gathers)


def _build(nc, cpsr, chof, CH, sched, gathers):
    x16d = nc.dram_tensor("x16", [P, HALF2], F16, kind="ExternalInput")
    xpsd = nc.dram_tensor("xps", [P, SHALF], F16, kind="ExternalInput")
    w1d = nc.dram_tensor("w1tp", [P, P], F16, kind="ExternalInput")
    wld = nc.dram_tensor("wlwr", [P, 2 * P], F16, kind="ExternalInput")
    cvd = nc.dram_tensor("cvec", [P, 4], F32, kind="ExternalInput")
    srd = nc.dram_tensor("srcw", [P, CH * 8], I16, kind="ExternalInput")
    dsd = nc.dram_tensor("dstw", [P, CH], F16, kind="ExternalInput")
    rcd = nc.dram_tensor("recn", [1, NODE_PAD], F16, kind="ExternalInput")
    mkd = nc.dram_tensor("msk", [P, P], F16, kind="ExternalInput")
    imd = nc.dram_tensor("idm", [P, P], F32, kind="ExternalInput")
    iod = nc.dram_tensor("iotas", [P, SUPB * P], F16, kind="ExternalInput")
    featd = nc.dram_tensor("featd", [P, SHARD], F16, kind="ExternalOutput")
    outfd = nc.dram_tensor("outfd", [P, SHARD], F16, kind="ExternalOutput")

    with tile.TileContext(nc) as tc, ExitStack() as ctx:
        persist = ctx.enter_context(tc.tile_pool(name="persist", bufs=1))
        dram = ctx.enter_context(tc.tile_pool(name="dram", bufs=1, space="DRAM"))
        xpool = ctx.enter_context(tc.tile_pool(name="xpool", bufs=3))
        twpool = ctx.enter_context(tc.tile_pool(name="twpool", bufs=3))
        gpool = ctx.enter_context(tc.tile_pool(name="gpool", bufs=28))
        ixpool = ctx.enter_context(tc.tile_pool(name="ixpool", bufs=3))
        ohpool = ctx.enter_context(tc.tile_pool(name="ohpool", bufs=8))
        opool = ctx.enter_context(tc.tile_pool(name="opool", bufs=3))
        psum = ctx.enter_context(tc.tile_pool(name="psum", bufs=1, space="PSUM"))

        table = dram.tile([NP2, P], F16)
        bn1_in = dram.tile([P, 2], F32)
        bn1_out = dram.tile([P, 2], F32, addr_space="Shared")
        bn2_in = dram.tile([P, 2], F32)
        bn2_out = dram.tile([P, 2], F32, addr_space="Shared")

        # ---- constants ----
        w1t16 = persist.tile([P, P], F16)
        nc.sync.dma_start(w1t16[:], w1d[:])
        wlwr16 = persist.tile([P, 2 * P], F16)
        nc.sync.dma_start(wlwr16[:], wld[:])
        cv = persist.tile([P, 4], F32)
        nc.sync.dma_start(cv[:], cvd[:])
        dstt = persist.tile([P, CH], F16)
        nc.scalar.dma_start(dstt[:], dsd[:])
        recn16 = persist.tile([1, NODE_PAD], F16)
        nc.sync.dma_start(recn16[:], rcd[:])
        msk16 = persist.tile([P, P], F16)
        nc.sync.dma_start(msk16[:], mkd[:])

        nc.gpsimd.load_library(mlp)
        id32 = persist.tile([P, P], F32)
        nc.sync.dma_start(id32[:], imd[:])
        ones1 = persist.tile([1, P], F16)
        nc.vector.memset(ones1[:], 1.0)
        # iotaS[:, b, c] = 128*b + c   (fp16, exact ints < 2048)
        iotaS = persist.tile([P, SUPB, P], F16)
        nc.sync.dma_start(iotaS[:], iod[:].rearrange("p (b c) -> p b c", b=SUPB))

        featT16 = persist.tile([P, NODE_PAD], F16)
        nc.vector.memset(featT16[:, SHARD:NODE_PAD], 0.0)
        aggT16 = persist.tile([P, NODE_PAD], F16)

        # ============ Stage 1a: own-shard raw s + BN1 partial stats ======
        SCH = (SHALF + 511) // 512  # 13
        st1 = persist.tile([P, 2 * SCH, 6], F32)
        for k in range(SCH):
            w = min(512, SHALF - 512 * k)
            xt = xpool.tile([P, 512], F16, tag="xt")
            nc.sync.dma_start(xt[:, :w], xpsd[:, 512 * k:512 * k + w])
            for h, (p0, slot) in enumerate(((0, 2 * k), (64, 2 * k + 1))):
                col0 = 512 * k + (0 if h == 0 else SHALF)
                ps = psum.tile([P, 512], F32, space="PSUM", tag="ps1", bufs=2)
                nc.tensor.matmul(ps[:, :w], lhsT=w1t16[p0:p0 + IN_DIM, :],
                                 rhs=xt[p0:p0 + IN_DIM, :w],
                                 start=True, stop=True)
                nc.vector.bn_stats(st1[:, slot:slot + 1, :], ps[:, :w])
                nc.scalar.copy(featT16[:, col0:col0 + w], ps[:, :w])

        # ---- BN1: local stats -> sum/sumsq -> AllReduce -> A1, B1 ----
        mv1 = persist.tile([P, 2], F32)
        nc.vector.bn_aggr(mv1[:], st1[:])
        ss1 = persist.tile([P, 2], F32)
        nc.scalar.mul(ss1[:, 0:1], mv1[:, 0:1], float(SHARD))
        nc.vector.tensor_tensor(ss1[:, 1:2], mv1[:, 0:1], mv1[:, 0:1],
                                op=ALU.mult)
        nc.vector.tensor_tensor(ss1[:, 1:2], mv1[:, 1:2], ss1[:, 1:2],
                                op=ALU.add)
        nc.vector.tensor_scalar_mul(ss1[:, 1:2], ss1[:, 1:2], float(SHARD))
        nc.sync.dma_start(bn1_in[:], ss1[:])
        nc.gpsimd.collective_compute(
            "AllReduce", ALU.add, replica_groups=[list(range(NCORES))],
            ins=[bn1_in[:]], outs=[bn1_out[:]])
        gst1 = persist.tile([P, 2], F32)
        nc.sync.dma_start(gst1[:], bn1_out[:])

        stat1 = persist.tile([P, 8], F32)
        nc.scalar.mul(stat1[:, 2:3], gst1[:, 0:1], 1.0 / N_NODES)   # mu1
        nc.scalar.mul(stat1[:, 3:4], gst1[:, 1:2], 1.0 / N_NODES)   # E[s^2]
        nc.vector.tensor_tensor(stat1[:, 4:5], stat1[:, 2:3], stat1[:, 2:3],
                                op=ALU.mult)
        nc.vector.tensor_tensor(stat1[:, 4:5], stat1[:, 3:4], stat1[:, 4:5],
                                op=ALU.subtract)
        nc.vector.tensor_scalar_add(stat1[:, 4:5], stat1[:, 4:5], BN_EPS)
        nc.vector.reciprocal(stat1[:, 5:6], stat1[:, 4:5])
        nc.scalar.sqrt(stat1[:, 5:6], stat1[:, 5:6])                # rsqrt
        A1 = persist.tile([P, 2], F32)   # col0 = A1, col1 = B1
        nc.vector.tensor_tensor(A1[:, 0:1], stat1[:, 5:6], cv[:, 0:1],
                                op=ALU.mult)
        nc.vector.tensor_tensor(A1[:, 1:2], stat1[:, 2:3], A1[:, 0:1],
                                op=ALU.mult)
        nc.vector.tensor_tensor(A1[:, 1:2], cv[:, 1:2], A1[:, 1:2],
                                op=ALU.subtract)

        # featT16 <- relu(A1 * s + B1) in place; write featd
        for k in range((SHARD + 511) // 512):
            w = min(512, SHARD - 512 * k)
            sl = slice(512 * k, 512 * k + w)
            nc.scalar.activation(featT16[:, sl], featT16[:, sl], ACTF.Relu,
                                 bias=A1[:, 1:2], scale=A1[:, 0:1])
        nc.sync.dma_start(featd[:], featT16[:, 0:SHARD])

        # ---- transpose [A1|B1] -> rows, broadcast, fold into weights ----
        arow_ps = psum.tile([1, P], F32, space="PSUM", tag="pc", bufs=2)
        nc.tensor.matmul(arow_ps[:], lhsT=A1[:, 0:1], rhs=id32[:],
                         is_transpose=True, start=True, stop=True)
        brow_ps = psum.tile([1, P], F32, space="PSUM", tag="pc", bufs=2)
        nc.tensor.matmul(brow_ps[:], lhsT=A1[:, 1:2], rhs=id32[:],
                         is_transpose=True, start=True, stop=True)
        arow16 = persist.tile([1, P], F16)
        brow16 = persist.tile([1, P], F16)
        nc.vector.tensor_copy(arow16[:], arow_ps[:])
        nc.vector.tensor_copy(brow16[:], brow_ps[:])
        a1b_ps = psum.tile([P, P], F32, space="PSUM", tag="pc", bufs=2)
        nc.tensor.matmul(a1b_ps[:], lhsT=ones1[0:1, :], rhs=arow16[:],
                         start=True, stop=True)
        b1b_ps = psum.tile([P, P], F32, space="PSUM", tag="pc", bufs=2)
        nc.tensor.matmul(b1b_ps[:], lhsT=ones1[0:1, :], rhs=brow16[:],
                         start=True, stop=True)
        # w1ts = W1T*A1 on data rows; B1 on the two ones-rows (msk selects).
        w1ts = persist.tile([P, P], F16)
        nc.vector.tensor_tensor(w1ts[:], w1t16[:], a1b_ps[:], op=ALU.mult)
        b1m = persist.tile([P, P], F16)
        nc.vector.tensor_tensor(b1m[:], msk16[:], b1b_ps[:], op=ALU.mult)
        nc.vector.tensor_tensor(w1ts[:], w1ts[:], b1m[:], op=ALU.add)

        # ============ Stage 1b: fp16 table, node-major ============
        for g in range(NGRP):
            h = 0 if g < NGRP // 2 else 64
            c0 = 1024 * (g if h == 0 else g - NGRP // 2)
            xg = xpool.tile([P, 1024], F16, tag="xg")
            nc.sync.dma_start(xg[:], x16d[:, c0:c0 + 1024])
            tw = twpool.tile([P, 8, P], F16, tag="tw")
            for i in range(8):
                pc = psum.tile([P, P], F32, space="PSUM", tag="pc", bufs=2)
                nc.tensor.matmul(pc[:], lhsT=xg[h:h + IN_DIM + 1,
                                                128 * i:128 * (i + 1)],
                                 rhs=w1ts[h:h + IN_DIM + 1, :],
                                 start=True, stop=True)
                nc.vector.tensor_scalar_max(tw[:, i, :], pc[:], 0.0)
            eng = nc.scalar if g % 2 else nc.sync
            eng.dma_start(
                table[1024 * g:1024 * (g + 1), :].rearrange(
                    "(p i) f -> p i f", p=P, i=8),
                tw[:])

        # ============ Stage 2: dma_gather + one-hot aggregation ======
        # recb16[p, n] = 1/deg(n) broadcast across partitions.
        recb16 = persist.tile([P, NODE_PAD], F16)
        for k in range((NODE_PAD + 511) // 512):
            w = min(512, NODE_PAD - 512 * k)
            rp = psum.tile([P, 512], F32, space="PSUM", tag="ps1", bufs=2)
            nc.tensor.matmul(rp[:, :w], lhsT=ones1[0:1, :],
                             rhs=recn16[0:1, 512 * k:512 * k + w],
                             start=True, stop=True)
            nc.scalar.copy(recb16[:, 512 * k:512 * k + w], rp[:, :w])

        # group gathers by super for interleaving with the matmul schedule
        from collections import defaultdict
        gby = defaultdict(list)
        for i, (s, r, c0, k, cc) in enumerate(gathers):
            gby[s].append((i, r, c0, k, cc))

        gtiles = {}     # first_chunk_col -> (tile, k)
        # per-super idx columns: [8*chof[s][0], 8*(chof[s][0]+sum_r cpsr[s]))
        scol0 = [8 * int(chof[s][0]) for s in range(NSUP)]
        swid = [8 * int(cpsr[s].sum()) for s in range(NSUP)]
        IXW = max(swid)

        def issue_gathers(s):
            if swid[s] == 0:
                return
            ixt = ixpool.tile([P, IXW], I16, tag="ix")
            nc.sync.dma_start(ixt[:, :swid[s]],
                              srd[:, scol0[s]:scol0[s] + swid[s]])
            for (i, r, c0, k, cc) in gby[s]:
                lc = c0 - scol0[s]
                gt = gpool.tile([P, 8, P], F16, tag="gt")
                nc.gpsimd.dma_gather(
                    gt[:, :k, :], table[RBASE[r]:RBASE[r] + RSIZE[r], :],
                    ixt[:, lc:lc + 8 * k], k * P, k * P, P,
                    queue_num=i % NQ)
                gtiles[cc] = (gt, k)

        def chunk_lhsT(col, s):
            # find gather tile containing chunk col
            for cc, (gt, k) in gtiles.items():
                if cc <= col < cc + k:
                    return gt[:, col - cc, :]
            raise KeyError(col)

        issue_gathers(0)
        for s in range(NSUP):
            items = sched[s]
            nblk_s = min(SUPB, NBLK - SUPB * s)
            # cols touching each block (in item order)
            touches = {b: [] for b in range(nblk_s)}
            for (col, r, lo, hi) in items:
                for b in range(lo, hi + 1):
                    touches[b].append(col)
            # 4 block-accumulators packed per [P,512] psum bank-tile.
            # PSUM accumulation chains must be CONTIGUOUS per region (an
            # interleaved start on a sibling region corrupts open chains),
            # so matmuls are emitted block-major.
            bpsA = psum.tile([P, 4 * P], F32, space="PSUM", tag="bps",
                             bufs=4, name=f"bpsA{s}")
            bpsB = (psum.tile([P, 4 * P], F32, space="PSUM", tag="bps",
                              bufs=4, name=f"bpsB{s}")
                    if nblk_s > 4 else None)

            def breg(b):
                t_ = bpsA if b < 4 else bpsB
                o = (b % 4) * P
                return t_[:, o:o + P]

            for b in range(nblk_s):
                cols = touches[b]
                cb = SUPB * s + b
                if not cols:            # untouched block: zero agg
                    nc.vector.memset(aggT16[:, P * cb:P * (cb + 1)], 0.0)
                    continue
                for t, col in enumerate(cols):
                    oh = ohpool.tile([P, P], F16, tag="oh")
                    nc.vector.tensor_tensor(
                        oh[:], dstt[:, col:col + 1].to_broadcast([P, P]),
                        iotaS[:, b, :], op=ALU.is_equal)
                    nc.tensor.matmul(breg(b), lhsT=chunk_lhsT(col, s),
                                     rhs=oh[:],
                                     start=(t == 0),
                                     stop=(t == len(cols) - 1))
                nc.vector.tensor_tensor(
                    aggT16[:, P * cb:P * (cb + 1)], breg(b),
                    recb16[:, P * cb:P * (cb + 1)], op=ALU.mult)
            # drop gather tiles of this super; issue the next super's
            # gathers only now, AFTER their pool-slot consumers exist.
            for (i, r, c0, k, cc) in gby[s]:
                gtiles.pop(cc, None)
            if s + 1 < NSUP:
                issue_gathers(s + 1)

        # ================= Stage 3: sage + BN2 =================
        # sage written back into aggT16 columns (read-then-write per chunk)
        NSCH = (NODE_PAD + 511) // 512  # 25 chunks (last = 256)
        st2 = persist.tile([P, NSCH, 6], F32)
        for k in range(NSCH):
            w = min(512, NODE_PAD - 512 * k)
            ws = min(512, max(0, SHARD - 512 * k))   # stats over 12500 only
            ps = psum.tile([P, 512], F32, space="PSUM", tag="ps1", bufs=2)
            nc.tensor.matmul(ps[:, :w], lhsT=wlwr16[:, 0:P],
                             rhs=aggT16[:, 512 * k:512 * k + w],
                             start=True, stop=False)
            nc.tensor.matmul(ps[:, :w], lhsT=wlwr16[:, P:2 * P],
                             rhs=featT16[:, 512 * k:512 * k + w],
                             start=False, stop=True)
            if ws > 0:
                nc.vector.bn_stats(st2[:, k:k + 1, :], ps[:, :ws])
            nc.scalar.copy(aggT16[:, 512 * k:512 * k + w], ps[:, :w])

        mv2 = persist.tile([P, 2], F32)
        nc.vector.bn_aggr(mv2[:], st2[:])
        ss2 = persist.tile([P, 2], F32)
        nc.scalar.mul(ss2[:, 0:1], mv2[:, 0:1], float(SHARD))
        nc.vector.tensor_tensor(ss2[:, 1:2], mv2[:, 0:1], mv2[:, 0:1],
                                op=ALU.mult)
        nc.vector.tensor_tensor(ss2[:, 1:2], mv2[:, 1:2], ss2[:, 1:2],
                                op=ALU.add)
        nc.vector.tensor_scalar_mul(ss2[:, 1:2], ss2[:, 1:2], float(SHARD))
        nc.sync.dma_start(bn2_in[:], ss2[:])
        nc.gpsimd.collective_compute(
            "AllReduce", ALU.add, replica_groups=[list(range(NCORES))],
            ins=[bn2_in[:]], outs=[bn2_out[:]])
        gst2 = persist.tile([P, 2], F32)
        nc.sync.dma_start(gst2[:], bn2_out[:])

        stat2 = persist.tile([P, 8], F32)
        nc.scalar.mul(stat2[:, 2:3], gst2[:, 0:1], 1.0 / N_NODES)   # mu2
        nc.scalar.mul(stat2[:, 3:4], gst2[:, 1:2], 1.0 / N_NODES)   # E[s^2]
        nc.vector.tensor_tensor(stat2[:, 4:5], stat2[:, 2:3], stat2[:, 2:3],
                                op=ALU.mult)
        nc.vector.tensor_tensor(stat2[:, 4:5], stat2[:, 3:4], stat2[:, 4:5],
                                op=ALU.subtract)
        nc.vector.tensor_scalar_add(stat2[:, 4:5], stat2[:, 4:5], BN_EPS)
        nc.vector.reciprocal(stat2[:, 5:6], stat2[:, 4:5])
        nc.scalar.sqrt(stat2[:, 5:6], stat2[:, 5:6])
        A2 = persist.tile([P, 2], F32)
        nc.vector.tensor_tensor(A2[:, 0:1], stat2[:, 5:6], cv[:, 2:3],
                                op=ALU.mult)
        nc.vector.tensor_tensor(A2[:, 1:2], stat2[:, 2:3], A2[:, 0:1],
                                op=ALU.mult)
        nc.vector.tensor_tensor(A2[:, 1:2], cv[:, 3:4], A2[:, 1:2],
                                op=ALU.subtract)

        for k in range((SHARD + 511) // 512):
            ws = min(512, SHARD - 512 * k)
            ot = opool.tile([P, 512], F16, tag="ot")
            nc.scalar.activation(ot[:, :ws], aggT16[:, 512 * k:512 * k + ws],
                                 ACTF.Identity, bias=A2[:, 1:2],
                                 scale=A2[:, 0:1])
            nc.sync.dma_start(outfd[:, 512 * k:512 * k + ws], ot[:, :ws])


def kernel(**inputs):
    x = np.asarray(inputs["x"], np.float32)
    edge_index = np.asarray(inputs["edge_index"])
    args = [x, edge_index,
            np.asarray(inputs["W1"], np.float32),
            np.asarray(inputs["Wl"], np.float32),
            np.asarray(inputs["Wr"], np.float32),
            np.asarray(inputs["g1"], np.float32),
            np.asarray(inputs["be1"], np.float32),
            np.asarray(inputs["g2"], np.float32),
            np.asarray(inputs["be2"], np.float32)]
    (x16, w1tp, msk, wlwr, cvec, idm, iotas, srcw, dstw, recn, xps,
     cpsr, chof, CH, sched, gathers) = _host_prep(*args)

    nc = bacc.Bacc("TRN2", target_bir_lowering=False, debug=False,
                   num_devices=NCORES, num_swdge_queues=NQ)
    _build(nc, cpsr, chof, CH, sched, gathers)
    nc.compile()

    in_maps = []
    for c in range(NCORES):
        in_maps.append({
            "x16": x16, "xps": np.ascontiguousarray(xps[c]),
            "w1tp": w1tp, "msk": msk, "wlwr": wlwr, "cvec": cvec,
            "idm": idm, "iotas": iotas.reshape(P, SUPB * P),
            "srcw": np.ascontiguousarray(srcw[c]),
            "dstw": np.ascontiguousarray(dstw[c]),
            "recn": np.ascontiguousarray(recn[c]),
        })
    res = run_bass_kernel_spmd(nc, in_maps, core_ids=list(range(NCORES)))
    feat = np.concatenate(
        [res.results[c]["featd"].astype(np.float32).T for c in range(NCORES)],
        axis=0)
    out_feat = np.concatenate(
        [res.results[c]["outfd"].astype(np.float32).T for c in range(NCORES)],
        axis=0)
    return (np.ascontiguousarray(feat), np.ascontiguousarray(out_feat))


# revision 21
# speedup vs baseline: 3.1465x; 1.3066x over previous
"""Trainium2 Bass kernel for CustomStellarEncoder (GNN message passing).

8 NeuronCores, dst-sharded graph parallelism, v3 (dma_gather rewrite):
  - Stage 1a (per-core): own-shard s = x @ W1.T in [feat, node] layout from
    xps16; bn_stats on psum, raw s stashed fp16 into featT16. BN1 stats
    (sum/sumsq) AllReduced across cores; A1/B1 finalized; featT16 updated
    in place to relu(A1*s + B1); featd written fp16.
  - Stage 1b (replicated): full-node fp16 gather table [100352, 128] built
    NODE-major (table row == node id); BN1 folded into weights, relu on DVE.
  - Stage 2 (dst-sharded): edges sorted by (super=dst//1024, range=src>>15,
    dst). Gathers via InstDMAGatherAnt (mlp ucode): 1024-idx int16 gathers
    round-robined over 4 SWDGE queues (per-queue descriptor ring budget
    ~64K; no mid-kernel reclaim on HW). Per 128-edge chunk: one-hot(dst)
    built by DVE is_equal against iota, matmul accumulates psum[feat, dst]
    per 128-dst block; supers of 8 blocks keep 8 psums live across the 4
    range-subruns. Finalize: aggT16 = psum * recip-degree broadcast.
  - Stage 3: sageT = WlT.T @ aggT16 + WrT.T @ featT16 per 512 cols (fp16),
    BN2 stats with cross-core AllReduce, out = A2*sage + B2 fp16 (sage
    written back into aggT16 columns to save SBUF).

b1 and bl are dropped: both cancel exactly under the following BatchNorm.
Outputs are fp16 [feat, node] per shard; host casts/transposes/concats.
"""

from contextlib import ExitStack

import numpy as np

import concourse.bass as bass
import concourse.tile as tile
from concourse import bacc, mybir
from concourse.bass_utils import run_bass_kernel_spmd
from concourse.library_config import mlp

N_NODES = 100000
N_EDGES = 1600000
IN_DIM = 48
HID = 128
BN_EPS = 1e-5
NCORES = 8
SHARD = N_NODES // NCORES          # 12500
P = 128
NBLK = (SHARD + P - 1) // P        # 98
NODE_PAD = NBLK * P                # 12544
NGRP = 98                          # 1024-node groups in padded table
NP2 = NGRP * 1024                  # 100352 padded table rows
HALF2 = NP2 // 2                   # 50176 (= 49 groups, group-aligned)
SHALF = SHARD // 2                 # 6250
F32 = mybir.dt.float32
F16 = mybir.dt.float16
I16 = mybir.dt.int16
AX = mybir.AxisListType
ALU = mybir.AluOpType
ACTF = mybir.ActivationFunctionType

SUPB = 8                            # dst blocks per super
NSUP = (NBLK + SUPB - 1) // SUPB    # 13 (last super has 2 blocks)
NRNG = 4
RBASE = [0, 32768, 65536, 98304]
RSIZE = [32768, 32768, 32768, NP2 - 98304]
GIDX = 1024                         # idxs per full dma_gather (8 chunks)
NQ = 4                              # SWDGE queues


def _host_prep(x, edge_index, W1, Wl, Wr, g1, be1, g2, be2):
    # ---- x16: [128, HALF2] fp16, 2 bands of 48 + ones rows, columns
    # permuted so chunk i / partition p of group g is node 1024g + 8p + i.
    xpad = np.zeros((NP2, IN_DIM), np.float32)
    xpad[:N_NODES] = x
    j = np.arange(1024)
    nig = 8 * (j % P) + j // P                      # node-in-group per col
    perm = (np.arange(0, NP2, 1024)[:, None] + nig[None, :]).ravel()
    xperm = xpad[perm]                              # [NP2, 48] col-ordered
    x16 = np.zeros((P, HALF2), np.float16)
    x16[0:IN_DIM] = xperm[:HALF2].T
    x16[IN_DIM] = 1.0
    x16[64:64 + IN_DIM] = xperm[HALF2:].T
    x16[64 + IN_DIM] = 1.0

    w1tp = np.zeros((P, P), np.float16)
    w1tp[0:IN_DIM] = W1.T
    w1tp[64:64 + IN_DIM] = W1.T

    msk = np.zeros((P, P), np.float16)
    msk[IN_DIM] = 1.0
    msk[64 + IN_DIM] = 1.0

    wlwr = np.zeros((P, 2 * P), np.float16)
    wlwr[:, 0:P] = Wl.T
    wlwr[:, P:2 * P] = Wr.T

    cvec = np.zeros((P, 4), np.float32)
    cvec[:, 0] = g1
    cvec[:, 1] = be1
    cvec[:, 2] = g2
    cvec[:, 3] = be2

    idm = np.eye(P, dtype=np.float32)
    iotas = np.zeros((P, SUPB, P), np.float16)
    iotas[:] = (128 * np.arange(SUPB)[:, None]
                + np.arange(P)[None, :]).astype(np.float16)[None, :, :]

    # ---- edges sorted by (core, super, range, dst) ----
    src = np.asarray(edge_index[0], np.int64)
    dst = np.asarray(edge_index[1], np.int64)
    core_of = dst // SHARD
    rng_of = src >> 15                    # 0..3 (98304.. -> 3)
    dl_all = dst - core_of * SHARD
    sup_of = dl_all // 1024
    key = ((core_of * NSUP + sup_of) * NRNG + rng_of) * (1 << 17) + dl_all
    order = np.argsort(key, kind="stable")
    src_s, dst_s, core_s = src[order], dst[order], core_of[order]
    rng_s, sup_s = rng_of[order], sup_of[order]
    dl_s = dl_all[order]

    core_starts = np.searchsorted(core_s, np.arange(NCORES + 1))

    # per (core, super, range) edge counts
    ncsr = np.zeros((NCORES, NSUP, NRNG), np.int64)
    flat = (core_s * NSUP + sup_s) * NRNG + rng_s
    bc = np.bincount(flat, minlength=NCORES * NSUP * NRNG)
    ncsr = bc.reshape(NCORES, NSUP, NRNG)
    cpsr = np.maximum(0, (ncsr.max(axis=0) + P - 1) // P)   # [NSUP, NRNG]
    CH = int(cpsr.sum())                                     # total chunks
    chof = np.zeros((NSUP, NRNG), np.int64)
    acc = 0
    for s in range(NSUP):
        for r in range(NRNG):
            chof[s, r] = acc
            acc += int(cpsr[s, r])

    # per-core wrapped int16 idxs + fp16 dst values (pad: idx 0, dst -1)
    srcw = np.zeros((NCORES, P, CH * 8), np.int16)
    dstw = np.full((NCORES, P, CH), -1.0, np.float16)
    # per-core per (s,r,chunk) block windows for union schedule
    blo = np.full((NSUP, NRNG, int(cpsr.max()) if CH else 1, ), 99, np.int64)
    bhi = np.full_like(blo, -1)
    recn = np.zeros((NCORES, 1, NODE_PAD), np.float16)
    xps = np.zeros((NCORES, P, SHALF), np.float16)
    for c in range(NCORES):
        s0, e0 = int(core_starts[c]), int(core_starts[c + 1])
        dl_c = dl_s[s0:e0]
        cnt = np.bincount(dl_c, minlength=NODE_PAD).astype(np.float32)
        recn[c, 0] = (1.0 / np.maximum(cnt, 1.0)).astype(np.float16)
        base = c * SHARD
        xps[c, 0:IN_DIM] = x[base:base + SHALF].T
        xps[c, 64:64 + IN_DIM] = x[base + SHALF:base + SHARD].T
        # run boundaries within this core
        sub = (sup_s[s0:e0] * NRNG + rng_s[s0:e0])
        starts = np.searchsorted(sub, np.arange(NSUP * NRNG + 1))
        for s in range(NSUP):
            for r in range(NRNG):
                a = int(starts[s * NRNG + r])
                b = int(starts[s * NRNG + r + 1])
                n = b - a
                if n == 0:
                    continue
                co = int(chof[s, r])
                i = np.arange(n)
                sv = (src_s[s0 + a:s0 + b] - RBASE[r]).astype(np.int16)
                dv = (dl_s[s0 + a:s0 + b] - 1024 * s).astype(np.float16)
                # idx wrap: idx position i -> [i % 16 (+16g), 8*co + i//16]
                colw = 8 * co + i // 16
                srcw[c, i % 16, colw] = sv
                dstw[c, i % P, co + i // P] = dv
                # block windows per chunk
                bb = (dl_s[s0 + a:s0 + b] - 1024 * s) // P
                for jj in range(int((n + P - 1) // P)):
                    seg = bb[jj * P:(jj + 1) * P]
                    blo[s, r, jj] = min(blo[s, r, jj], int(seg.min()))
                    bhi[s, r, jj] = max(bhi[s, r, jj], int(seg.max()))
    # replicate idx wrap across the 8 gpsimd core groups
    for g in range(1, 8):
        srcw[:, 16 * g:16 * g + 16, :] = srcw[:, 0:16, :]

    # ---- static schedule ----
    # per super: ordered list over (r, j) of (chunk_col, r, window lo, hi)
    # plus start/stop bookkeeping per block.
    sched = []          # [NSUP] -> list of (col, r, lo, hi)
    for s in range(NSUP):
        items = []
        for r in range(NRNG):
            for jj in range(int(cpsr[s, r])):
                lo, hi = int(blo[s, r, jj]), int(bhi[s, r, jj])
                if hi < 0:      # no core has edges in this chunk (all pad)
                    lo = hi = 0  # harmless zero matmul into block 0
                items.append((int(chof[s, r]) + jj, r, lo, hi))
        sched.append(items)

    # gather split per (s, r): list of (idx_col0, nchunks, first_chunk_col)
    gathers = []
    for s in range(NSUP):
        for r in range(NRNG):
            nch = int(cpsr[s, r])
            co = int(chof[s, r])
            jj = 0
            while jj < nch:
                k = min(8, nch - jj)
                gathers.append((s, r, 8 * (co + jj), k, co + jj))
                jj += k
    # ring budget check: pow2ceil(k) slot pages per gather, <=512/queue
    def p2(k):
        v = 1
        while v < k:
            v *= 2
        return v
    pages = [0] * NQ
    for i, (s, r, c0, k, cc) in enumerate(gathers):
        pages[i % NQ] += p2(k)
    assert max(pages) <= 448, f"SWDGE ring budget exceeded: {pages}"

    return (x16, w1tp, msk, wlwr, cvec, idm, iotas, srcw, dstw, recn, xps,
            cpsr, chof, CH, sched, gathers)


def _build(nc, cpsr, chof, CH, sched, gathers):
    x16d = nc.dram_tensor("x16", [P, HALF2], F16, kind="ExternalInput")
    xpsd = nc.dram_tensor("xps", [P, SHALF], F16, kind="ExternalInput")
    w1d = nc.dram_tensor("w1tp", [P, P], F16, kind="ExternalInput")
    wld = nc.dram_tensor("wlwr", [P, 2 * P], F16, kind="ExternalInput")
    cvd = nc.dram_tensor("cvec", [P, 4], F32, kind="ExternalInput")
    srd = nc.dram_tensor("srcw", [P, CH * 8], I16, kind="ExternalInput")
    dsd = nc.dram_tensor("dstw", [P, CH], F16, kind="ExternalInput")
    rcd = nc.dram_tensor("recn", [1, NODE_PAD], F16, kind="ExternalInput")
    mkd = nc.dram_tensor("msk", [P, P], F16, kind="ExternalInput")
    imd = nc.dram_tensor("idm", [P, P], F32, kind="ExternalInput")
    iod = nc.dram_tensor("iotas", [P, SUPB * P], F16, kind="ExternalInput")
    featd = nc.dram_tensor("featd", [P, SHARD], F16, kind="ExternalOutput")
    outfd = nc.dram_tensor("outfd", [P, SHARD], F16, kind="ExternalOutput")

    with tile.TileContext(nc) as tc, ExitStack() as ctx:
        persist = ctx.enter_context(tc.tile_pool(name="persist", bufs=1))
        dram = ctx.enter_context(tc.tile_pool(name="dram", bufs=1, space="DRAM"))
        xpool = ctx.enter_context(tc.tile_pool(name="xpool", bufs=3))
        twpool = ctx.enter_context(tc.tile_pool(name="twpool", bufs=3))
        gpool = ctx.enter_context(tc.tile_pool(name="gpool", bufs=28))
        ixpool = ctx.enter_context(tc.tile_pool(name="ixpool", bufs=3))
        ohpool = ctx.enter_context(tc.tile_pool(name="ohpool", bufs=8))
        opool = ctx.enter_context(tc.tile_pool(name="opool", bufs=3))
        psum = ctx.enter_context(tc.tile_pool(name="psum", bufs=1, space="PSUM"))

        table = dram.tile([NP2, P], F16)
        bn1_in = dram.tile([P, 2], F32)
        bn1_out = dram.tile([P, 2], F32, addr_space="Shared")
        bn2_in = dram.tile([P, 2], F32)
        bn2_out = dram.tile([P, 2], F32, addr_space="Shared")

        # ---- constants ----
        w1t16 = persist.tile([P, P], F16)
        nc.sync.dma_start(w1t16[:], w1d[:])
        wlwr16 = persist.tile([P, 2 * P], F16)
        nc.sync.dma_start(wlwr16[:], wld[:])
        cv = persist.tile([P, 4], F32)
        nc.sync.dma_start(cv[:], cvd[:])
        dstt = persist.tile([P, CH], F16)
        nc.scalar.dma_start(dstt[:], dsd[:])
        recn16 = persist.tile([1, NODE_PAD], F16)
        nc.sync.dma_start(recn16[:], rcd[:])
        msk16 = persist.tile([P, P], F16)
        nc.sync.dma_start(msk16[:], mkd[:])

        nc.gpsimd.load_library(mlp)
        id32 = persist.tile([P, P], F32)
        nc.sync.dma_start(id32[:], imd[:])
        ones1 = persist.tile([1, P], F16)
        nc.vector.memset(ones1[:], 1.0)
        # iotaS[:, b, c] = 128*b + c   (fp16, exact ints < 2048)
        iotaS = persist.tile([P, SUPB, P], F16)
        nc.sync.dma_start(iotaS[:], iod[:].rearrange("p (b c) -> p b c", b=SUPB))

        featT16 = persist.tile([P, NODE_PAD], F16)
        nc.vector.memset(featT16[:, SHARD:NODE_PAD], 0.0)
        aggT16 = persist.tile([P, NODE_PAD], F16)

        # ============ Stage 1a: own-shard raw s + BN1 partial stats ======
        SCH = (SHALF + 511) // 512  # 13
        st1 = persist.tile([P, 2 * SCH, 6], F32)
        for k in range(SCH):
            w = min(512, SHALF - 512 * k)
            xt = xpool.tile([P, 512], F16, tag="xt")
            nc.sync.dma_start(xt[:, :w], xpsd[:, 512 * k:512 * k + w])
            for h, (p0, slot) in enumerate(((0, 2 * k), (64, 2 * k + 1))):
                col0 = 512 * k + (0 if h == 0 else SHALF)
                ps = psum.tile([P, 512], F32, space="PSUM", tag="ps1", bufs=2)
                nc.tensor.matmul(ps[:, :w], lhsT=w1t16[p0:p0 + IN_DIM, :],
                                 rhs=xt[p0:p0 + IN_DIM, :w],
                                 start=True, stop=True)
                nc.vector.bn_stats(st1[:, slot:slot + 1, :], ps[:, :w])
                nc.scalar.copy(featT16[:, col0:col0 + w], ps[:, :w])

        # ---- BN1: local stats -> sum/sumsq -> AllReduce -> A1, B1 ----
        mv1 = persist.tile([P, 2], F32)
        nc.vector.bn_aggr(mv1[:], st1[:])
        ss1 = persist.tile([P, 2], F32)
        nc.scalar.mul(ss1[:, 0:1], mv1[:, 0:1], float(SHARD))
        nc.vector.tensor_tensor(ss1[:, 1:2], mv1[:, 0:1], mv1[:, 0:1],
                                op=ALU.mult)
        nc.vector.tensor_tensor(ss1[:, 1:2], mv1[:, 1:2], ss1[:, 1:2],
                                op=ALU.add)
        nc.vector.tensor_scalar_mul(ss1[:, 1:2], ss1[:, 1:2], float(SHARD))
        nc.sync.dma_start(bn1_in[:], ss1[:])
        nc.gpsimd.collective_compute(
            "AllReduce", ALU.add, replica_groups=[list(range(NCORES))],
            ins=[bn1_in[:]], outs=[bn1_out[:]])
        gst1 = persist.tile([P, 2], F32)
        nc.sync.dma_start(gst1[:], bn1_out[:])

        stat1 = persist.tile([P, 8], F32)
        nc.scalar.mul(stat1[:, 2:3], gst1[:, 0:1], 1.0 / N_NODES)   # mu1
        nc.scalar.mul(stat1[:, 3:4], gst1[:, 1:2], 1.0 / N_NODES)   # E[s^2]
        nc.vector.tensor_tensor(stat1[:, 4:5], stat1[:, 2:3], stat1[:, 2:3],
                                op=ALU.mult)
        nc.vector.tensor_tensor(stat1[:, 4:5], stat1[:, 3:4], stat1[:, 4:5],
                                op=ALU.subtract)
        nc.vector.tensor_scalar_add(stat1[:, 4:5], stat1[:, 4:5], BN_EPS)
        nc.vector.reciprocal(stat1[:, 5:6], stat1[:, 4:5])
        nc.scalar.sqrt(stat1[:, 5:6], stat1[:, 5:6])                # rsqrt
        A1 = persist.tile([P, 2], F32)   # col0 = A1, col1 = B1
        nc.vector.tensor_tensor(A1[:, 0:1], stat1[:, 5:6], cv[:, 0:1],
                                op=ALU.mult)
        nc.vector.tensor_tensor(A1[:, 1:2], stat1[:, 2:3], A1[:, 0:1],
                                op=ALU.mult)
        nc.vector.tensor_tensor(A1[:, 1:2], cv[:, 1:2], A1[:, 1:2],
                                op=ALU.subtract)

        # featT16 <- relu(A1 * s + B1) in place; write featd
        for k in range((SHARD + 511) // 512):
            w = min(512, SHARD - 512 * k)
            sl = slice(512 * k, 512 * k + w)
            nc.scalar.activation(featT16[:, sl], featT16[:, sl], ACTF.Relu,
                                 bias=A1[:, 1:2], scale=A1[:, 0:1])
        nc.sync.dma_start(featd[:], featT16[:, 0:SHARD])

        # ---- transpose [A1|B1] -> rows, broadcast, fold into weights ----
        arow_ps = psum.tile([1, P], F32, space="PSUM", tag="pc", bufs=2)
        nc.tensor.matmul(arow_ps[:], lhsT=A1[:, 0:1], rhs=id32[:],
                         is_transpose=True, start=True, stop=True)
        brow_ps = psum.tile([1, P], F32, space="PSUM", tag="pc", bufs=2)
        nc.tensor.matmul(brow_ps[:], lhsT=A1[:, 1:2], rhs=id32[:],
                         is_transpose=True, start=True, stop=True)
        arow16 = persist.tile([1, P], F16)
        brow16 = persist.tile([1, P], F16)
        nc.vector.tensor_copy(arow16[:], arow_ps[:])
        nc.vector.tensor_copy(brow16[:], brow_ps[:])
        a1b_ps = psum.tile([P, P], F32, space="PSUM", tag="pc", bufs=2)
        nc.tensor.matmul(a1b_ps[:], lhsT=ones1[0:1, :], rhs=arow16[:],
                         start=True, stop=True)
        b1b_ps = psum.tile([P, P], F32, space="PSUM", tag="pc", bufs=2)
        nc.tensor.matmul(b1b_ps[:], lhsT=ones1[0:1, :], rhs=brow16[:],
                         start=True, stop=True)
        # w1ts = W1T*A1 on data rows; B1 on the two ones-rows (msk selects).
        w1ts = persist.tile([P, P], F16)
        nc.vector.tensor_tensor(w1ts[:], w1t16[:], a1b_ps[:], op=ALU.mult)
        b1m = persist.tile([P, P], F16)
        nc.vector.tensor_tensor(b1m[:], msk16[:], b1b_ps[:], op=ALU.mult)
        nc.vector.tensor_tensor(w1ts[:], w1ts[:], b1m[:], op=ALU.add)

        # ============ Stage 1b: fp16 table, node-major ============
        # One xg load per column group serves BOTH bands (nodes 1024g and
        # 1024(g+49)) — halves x16 read traffic.
        for g in range(NGRP // 2):
            xg = xpool.tile([P, 1024], F16, tag="xg")
            nc.sync.dma_start(xg[:], x16d[:, 1024 * g:1024 * (g + 1)])
            for h, gg in ((0, g), (64, g + NGRP // 2)):
                tw = twpool.tile([P, 8, P], F16, tag="tw")
                for i in range(8):
                    pc = psum.tile([P, P], F32, space="PSUM", tag="pc",
                                   bufs=2)
                    nc.tensor.matmul(pc[:], lhsT=xg[h:h + IN_DIM + 1,
                                                    128 * i:128 * (i + 1)],
                                     rhs=w1ts[h:h + IN_DIM + 1, :],
                                     start=True, stop=True)
                    nc.vector.tensor_scalar_max(tw[:, i, :], pc[:], 0.0)
                eng = nc.scalar if gg % 2 else nc.sync
                eng.dma_start(
                    table[1024 * gg:1024 * (gg + 1), :].rearrange(
                        "(p i) f -> p i f", p=P, i=8),
                    tw[:])

        # ============ Stage 2: dma_gather + one-hot aggregation ======
        # recb16[p, n] = 1/deg(n) broadcast across partitions.
        recb16 = persist.tile([P, NODE_PAD], F16)
        for k in range((NODE_PAD + 511) // 512):
            w = min(512, NODE_PAD - 512 * k)
            rp = psum.tile([P, 512], F32, space="PSUM", tag="ps1", bufs=2)
            nc.tensor.matmul(rp[:, :w], lhsT=ones1[0:1, :],
                             rhs=recn16[0:1, 512 * k:512 * k + w],
                             start=True, stop=True)
            nc.scalar.copy(recb16[:, 512 * k:512 * k + w], rp[:, :w])

        # group gathers by super for interleaving with the matmul schedule
        from collections import defaultdict
        gby = defaultdict(list)
        for i, (s, r, c0, k, cc) in enumerate(gathers):
            gby[s].append((i, r, c0, k, cc))

        # stage-3 chunk emitter (interleaved: chunk k ready once aggT cols
        # [512k, 512k+512) are finalized = after super k//2)
        NSCH = (NODE_PAD + 511) // 512  # 25 chunks (last = 256)
        st2 = persist.tile([P, NSCH, 6], F32)

        def emit_stage3(k):
            w = min(512, NODE_PAD - 512 * k)
            ws = min(512, max(0, SHARD - 512 * k))   # stats over 12500 only
            ps = psum.tile([P, 512], F32, space="PSUM", tag="ps1", bufs=2)
            nc.tensor.matmul(ps[:, :w], lhsT=wlwr16[:, 0:P],
                             rhs=aggT16[:, 512 * k:512 * k + w],
                             start=True, stop=False)
            nc.tensor.matmul(ps[:, :w], lhsT=wlwr16[:, P:2 * P],
                             rhs=featT16[:, 512 * k:512 * k + w],
                             start=False, stop=True)
            if ws > 0:
                nc.vector.bn_stats(st2[:, k:k + 1, :], ps[:, :ws])
            nc.scalar.copy(aggT16[:, 512 * k:512 * k + w], ps[:, :w])

        gtiles = {}     # first_chunk_col -> (tile, k)
        # per-super idx columns: [8*chof[s][0], 8*(chof[s][0]+sum_r cpsr[s]))
        scol0 = [8 * int(chof[s][0]) for s in range(NSUP)]
        swid = [8 * int(cpsr[s].sum()) for s in range(NSUP)]
        IXW = max(swid)

        def issue_gathers(s):
            if swid[s] == 0:
                return
            ixt = ixpool.tile([P, IXW], I16, tag="ix")
            nc.sync.dma_start(ixt[:, :swid[s]],
                              srd[:, scol0[s]:scol0[s] + swid[s]])
            for (i, r, c0, k, cc) in gby[s]:
                lc = c0 - scol0[s]
                gt = gpool.tile([P, 8, P], F16, tag="gt")
                nc.gpsimd.dma_gather(
                    gt[:, :k, :], table[RBASE[r]:RBASE[r] + RSIZE[r], :],
                    ixt[:, lc:lc + 8 * k], k * P, k * P, P,
                    queue_num=i % NQ)
                gtiles[cc] = (gt, k)

        def chunk_lhsT(col, s):
            # find gather tile containing chunk col
            for cc, (gt, k) in gtiles.items():
                if cc <= col < cc + k:
                    return gt[:, col - cc, :]
            raise KeyError(col)

        issue_gathers(0)
        for s in range(NSUP):
            items = sched[s]
            nblk_s = min(SUPB, NBLK - SUPB * s)
            # cols touching each block (in item order)
            touches = {b: [] for b in range(nblk_s)}
            for (col, r, lo, hi) in items:
                for b in range(lo, hi + 1):
                    touches[b].append(col)
            # 4 block-accumulators packed per [P,512] psum bank-tile.
            # PSUM accumulation chains must be CONTIGUOUS per region (an
            # interleaved start on a sibling region corrupts open chains),
            # so matmuls are emitted block-major.
            bpsA = psum.tile([P, 4 * P], F32, space="PSUM", tag="bps",
                             bufs=4, name=f"bpsA{s}")
            bpsB = (psum.tile([P, 4 * P], F32, space="PSUM", tag="bps",
                              bufs=4, name=f"bpsB{s}")
                    if nblk_s > 4 else None)

            def breg(b):
                t_ = bpsA if b < 4 else bpsB
                o = (b % 4) * P
                return t_[:, o:o + P]

            for b in range(nblk_s):
                cols = touches[b]
                cb = SUPB * s + b
                if not cols:            # untouched block: zero agg
                    nc.vector.memset(aggT16[:, P * cb:P * (cb + 1)], 0.0)
                    continue
                # split into consecutive-col spans capped at 8 (one DVE
                # is_equal per span instead of per chunk)
                spans = []
                for col in cols:
                    if (spans and col == spans[-1][0] + spans[-1][1]
                            and spans[-1][1] < 8):
                        spans[-1][1] += 1
                    else:
                        spans.append([col, 1])
                t = 0
                ncols = len(cols)
                for c0s, nj in spans:
                    oh = ohpool.tile([P, 8, P], F16, tag="oh")
                    nc.vector.tensor_tensor(
                        oh[:, :nj, :],
                        dstt[:, c0s:c0s + nj].to_broadcast([P, nj, P]),
                        iotaS[:, b:b + 1, :].to_broadcast([P, nj, P]),
                        op=ALU.is_equal)
                    for j in range(nj):
                        nc.tensor.matmul(breg(b),
                                         lhsT=chunk_lhsT(c0s + j, s),
                                         rhs=oh[:, j, :],
                                         start=(t == 0),
                                         stop=(t == ncols - 1))
                        t += 1
                nc.vector.tensor_tensor(
                    aggT16[:, P * cb:P * (cb + 1)], breg(b),
                    recb16[:, P * cb:P * (cb + 1)], op=ALU.mult)
            # drop gather tiles of this super; issue the next super's
            # gathers only now, AFTER their pool-slot consumers exist.
            for (i, r, c0, k, cc) in gby[s]:
                gtiles.pop(cc, None)
            if s + 1 < NSUP:
                issue_gathers(s + 1)
            for k3 in (2 * s, 2 * s + 1):
                if k3 < NSCH:
                    emit_stage3(k3)

        # ================= Stage 3 epilogue: BN2 =================
        mv2 = persist.tile([P, 2], F32)
        nc.vector.bn_aggr(mv2[:], st2[:])
        ss2 = persist.tile([P, 2], F32)
        nc.scalar.mul(ss2[:, 0:1], mv2[:, 0:1], float(SHARD))
        nc.vector.tensor_tensor(ss2[:, 1:2], mv2[:, 0:1], mv2[:, 0:1],
                                op=ALU.mult)
        nc.vector.tensor_tensor(ss2[:, 1:2], mv2[:, 1:2], ss2[:, 1:2],
                                op=ALU.add)
        nc.vector.tensor_scalar_mul(ss2[:, 1:2], ss2[:, 1:2], float(SHARD))
        nc.sync.dma_start(bn2_in[:], ss2[:])
        nc.gpsimd.collective_compute(
            "AllReduce", ALU.add, replica_groups=[list(range(NCORES))],
            ins=[bn2_in[:]], outs=[bn2_out[:]])
        gst2 = persist.tile([P, 2], F32)
        nc.sync.dma_start(gst2[:], bn2_out[:])

        stat2 = persist.tile([P, 8], F32)
        nc.scalar.mul(stat2[:, 2:3], gst2[:, 0:1], 1.0 / N_NODES)   # mu2
        nc.scalar.mul(stat2[:, 3:4], gst2[:, 1:2], 1.0 / N_NODES)   # E[s^2]
        nc.vector.tensor_tensor(stat2[:, 4:5], stat2[:, 2:3], stat2[:, 2:3],
                                op=ALU.mult)
        nc.vector.tensor_tensor(stat2[:, 4:5], stat2[:, 3:4], stat2[:, 4:5],
                                op=ALU.subtract)
        nc.vector.tensor_scalar_add(stat2[:, 4:5], stat2[:, 4:5], BN_EPS)
        nc.vector.reciprocal(stat2[:, 5:6], stat2[:, 4:5])
        nc.scalar.sqrt(stat2[:, 5:6], stat2[:, 5:6])
        A2 = persist.tile([P, 2], F32)
        nc.vector.tensor_tensor(A2[:, 0:1], stat2[:, 5:6], cv[:, 2:3],
                                op=ALU.mult)
        nc.vector.tensor_tensor(A2[:, 1:2], stat2[:, 2:3], A2[:, 0:1],
                                op=ALU.mult)
        nc.vector.tensor_tensor(A2[:, 1:2], cv[:, 3:4], A2[:, 1:2],
                                op=ALU.subtract)

        for k in range((SHARD + 511) // 512):
            ws = min(512, SHARD - 512 * k)
            ot = opool.tile([P, 512], F16, tag="ot")
            nc.scalar.activation(ot[:, :ws], aggT16[:, 512 * k:512 * k + ws],
                                 ACTF.Identity, bias=A2[:, 1:2],
                                 scale=A2[:, 0:1])
            nc.sync.dma_start(outfd[:, 512 * k:512 * k + ws], ot[:, :ws])


def kernel(**inputs):
    x = np.asarray(inputs["x"], np.float32)
    edge_index = np.asarray(inputs["edge_index"])
    args = [x, edge_index,
            np.asarray(inputs["W1"], np.float32),
            np.asarray(inputs["Wl"], np.float32),
            np.asarray(inputs["Wr"], np.float32),
            np.asarray(inputs["g1"], np.float32),
            np.asarray(inputs["be1"], np.float32),
            np.asarray(inputs["g2"], np.float32),
            np.asarray(inputs["be2"], np.float32)]
    (x16, w1tp, msk, wlwr, cvec, idm, iotas, srcw, dstw, recn, xps,
     cpsr, chof, CH, sched, gathers) = _host_prep(*args)

    nc = bacc.Bacc("TRN2", target_bir_lowering=False, debug=False,
                   num_devices=NCORES, num_swdge_queues=NQ)
    _build(nc, cpsr, chof, CH, sched, gathers)
    nc.compile()

    in_maps = []
    for c in range(NCORES):
        in_maps.append({
            "x16": x16, "xps": np.ascontiguousarray(xps[c]),
            "w1tp": w1tp, "msk": msk, "wlwr": wlwr, "cvec": cvec,
            "idm": idm, "iotas": iotas.reshape(P, SUPB * P),
            "srcw": np.ascontiguousarray(srcw[c]),
            "dstw": np.ascontiguousarray(dstw[c]),
            "recn": np.ascontiguousarray(recn[c]),
        })
    res = run_bass_kernel_spmd(nc, in_maps, core_ids=list(range(NCORES)))
    feat = np.concatenate(
        [res.results[c]["featd"].astype(np.float32).T for c in range(NCORES)],
        axis=0)
    out_feat = np.concatenate(
        [res.results[c]["outfd"].astype(np.float32).T for c in range(NCORES)],
        axis=0)
    return (np.ascontiguousarray(feat), np.ascontiguousarray(out_feat))
